# revision 18
# baseline (speedup 1.0000x reference)
"""Attention graph convolution (GAT layer) on 8 TRN2 NeuronCores.

Reference computation (all fp32):
    h   = input @ W                      # (N, 64)
    e   = leakyrelu(h@a1 + (h@a2).T)     # (N, N)
    att = softmax(where(adj>0, e, -inf)) # row softmax
    out = elu(att @ h)                   # (N, 64)

Sharding: rows of e/att (= output rows) are split across 8 cores,
1536 rows each.  h (N x 64) is computed on every core (tiny).

Per-core algorithm (core owns rows I, |I| = 1536):
  - no max-subtraction softmax: z values are small (|z| < ~30), so
    U[i,j] = adj[i,j] * exp(leakyrelu(Wh1_i + Wh2_j)) cannot overflow and
    equals the reference numerator up to the common exp(-max) factor.
  - denominator via ones-column: P = U @ [h | 1]; out = elu(P[:, :64] / P[:, 64])
  - U is built in TRANSPOSED layout [j partitions, i free] so it can feed
    the PE matmul (contraction dim = partition dim) with no U transpose:
        P.T[f, i] = sum_j h_ext[j, f] * U.T[j, i]
    adj row-blocks are DMA'd contiguously (int32 -> f32 cast in SWDGE) and
    transposed 128x128-at-a-time on the tensor engine into PSUM; the mask
    multiply reads adj.T directly from PSUM (no extra copy pass).
"""

import numpy as np

N_TOTAL = 12288
K_IN = 128
F_OUT = 64
N_CORES = 8
ALPHA = 0.2


def build_program(
    nt: int,          # total nodes (columns of adj)
    no: int,          # nodes owned by this core (rows of adj block)
    jw: int,          # j window size (columns resident in SBUF at once)
    u_bf16: bool = False,  # U / h_ext in bf16 for the big matmul
    lrelu_act_frac: float = 0.65,  # j-chunk fraction with leakyrelu on ACT
    lrelu_dve_frac: float = 0.35,  # ... on DVE (rest goes to GpSimd)
):
    from contextlib import ExitStack

    import concourse.bass as bass
    import concourse.mybir as mybir
    import concourse.tile as tile
    from concourse import bacc
    from concourse.alu_op_type import AluOpType
    from concourse.masks import make_identity

    f32 = mybir.dt.float32
    i32 = mybir.dt.int32
    bf16 = mybir.dt.bfloat16
    AF = mybir.ActivationFunctionType
    u_dt = bf16 if u_bf16 else f32

    P = 128
    F = F_OUT
    FE = F + 1                    # h columns + ones column
    K = K_IN
    assert nt % P == 0 and no % P == 0 and jw % P == 0 and nt % jw == 0
    ncj = nt // P                 # global j chunks
    nw = nt // jw                 # windows
    cpw = jw // P                 # j chunks per window
    nic = no // P                 # i chunks (own rows)
    S = 512                       # i split for matmul N-dim / psum banks
    ns = (no + S - 1) // S
    assert no % S == 0 or ns == 1

    nc = bacc.Bacc("TRN2", target_bir_lowering=False, debug=False,
                   num_devices=1)

    inp = nc.dram_tensor("input", [nt, K], f32, kind="ExternalInput")
    inp_own = nc.dram_tensor("input_own", [no, K], f32, kind="ExternalInput")
    adj_own = nc.dram_tensor("adj_own", [no, nt], i32, kind="ExternalInput")
    w_d = nc.dram_tensor("W", [K, F], f32, kind="ExternalInput")
    a_d = nc.dram_tensor("a", [2 * F, 1], f32, kind="ExternalInput")
    out_d = nc.dram_tensor("out", [no, F], f32, kind="ExternalOutput")

    with tile.TileContext(nc) as tc, ExitStack() as ctx:
        consts = ctx.enter_context(tc.tile_pool(name="consts", bufs=1))

        identity = consts.tile([P, P], f32)
        make_identity(nc, identity)
        identity_bf = consts.tile([P, P], bf16)
        nc.vector.tensor_copy(identity_bf[:], identity[:])

        # ---- phase 0: Wa1 = W @ a1, Wa2 = W @ a2 -------------------------
        wt_sb = consts.tile([F, K], f32)       # W.T  (64 x 128)
        nc.sync.dma_start(wt_sb[:], w_d.ap().rearrange("k f -> f k"))
        a_sb = consts.tile([F, 2], f32)        # [a1 | a2] (64 x 2)
        nc.sync.dma_start(a_sb[:], a_d.ap().rearrange("(n f) o -> f (n o)", n=2))
        wwa2_sb = consts.tile([K, FE], f32)    # [W | Wa2] (128 x 65)
        nc.sync.dma_start(wwa2_sb[:, 0:F], w_d.ap())

        wa12_sb = consts.tile([K, 2], f32)
        ones_sb = consts.tile([P, P], f32)
        nc.vector.memset(ones_sb[:], 1.0)
        wa1_rep = consts.tile([K, P], f32)     # Wa1 replicated to 128 cols

        with tc.tile_pool(name="ph0_psum", bufs=2, space="PSUM") as ph0_psum:
            wa_ps = ph0_psum.tile([K, 2], f32)
            nc.tensor.matmul(wa_ps[:], wt_sb[:], a_sb[:], start=True, stop=True)
            nc.vector.tensor_copy(wa12_sb[:], wa_ps[:])
        nc.vector.tensor_copy(wwa2_sb[:, F:FE], wa12_sb[:, 1:2])
        # wa1_rep[k, m] = Wa1[k] for all m
        nc.vector.tensor_scalar(wa1_rep[:], ones_sb[:], wa12_sb[:, 0:1], None,
                                AluOpType.mult)

        # ---- phase 1a: Wh1_rep[p, x] = Wh1[own x] for all p --------------
        # Wh1_rep = wa1_rep.T @ input_own.T ; input_own.T via PE transposes.
        wh1_rep = consts.tile([P, no], f32)
        into_sb = consts.tile([K, no], f32)    # input_own.T
        with (
            tc.tile_pool(name="ph1a_in", bufs=3) as pin,
            tc.tile_pool(name="ph1a_ps", bufs=3, space="PSUM") as pps,
        ):
            for ic in range(nic):
                ich = pin.tile([P, K], f32, tag="ich")
                nc.sync.dma_start(ich[:], inp_own[ic * P:(ic + 1) * P, :])
                itp = pps.tile([K, P], f32, tag="itp")
                nc.tensor.transpose(itp[:], ich[:], identity[:])
                nc.vector.tensor_copy(into_sb[:, ic * P:(ic + 1) * P], itp[:])
            for s in range(ns):
                sw = min(S, no - s * S)
                w1p = pps.tile([P, S], f32, tag="w1p")
                nc.tensor.matmul(w1p[:, 0:sw], wa1_rep[:],
                                 into_sb[:, s * S:s * S + sw],
                                 start=True, stop=True)
                nc.vector.tensor_copy(wh1_rep[:, s * S:s * S + sw],
                                      w1p[:, 0:sw])

        # ---- phase 1b: h_ext = [h | 1] (bf16) and Wh2 per j chunk --------
        h_ext = consts.tile([P, ncj, FE], u_dt)
        wh2_sb = consts.tile([P, ncj], f32)
        with (
            tc.tile_pool(name="ph1b_in", bufs=3) as pin,
            tc.tile_pool(name="ph1b_it", bufs=3) as pit,
            tc.tile_pool(name="ph1b_ps", bufs=4, space="PSUM") as pps,
        ):
            for jc in range(ncj):
                jch = pin.tile([P, K], f32, tag="jch")
                nc.sync.dma_start(jch[:], inp[jc * P:(jc + 1) * P, :])
                jtp = pps.tile([K, P], f32, tag="jtp")
                nc.tensor.transpose(jtp[:], jch[:], identity[:])
                jts = pit.tile([K, P], f32, tag="jts")
                nc.scalar.copy(jts[:], jtp[:])
                hw_ps = pps.tile([P, FE], f32, tag="hw")
                nc.tensor.matmul(hw_ps[:], jts[:], wwa2_sb[:],
                                 start=True, stop=True)
                nc.scalar.copy(h_ext[:, jc, 0:F], hw_ps[:, 0:F])
                nc.vector.tensor_copy(wh2_sb[:, jc:jc + 1], hw_ps[:, F:FE])
        nc.vector.memset(h_ext[:, :, F], 1.0)

        # ---- phase 2: main loop over j windows / j chunks ----------------
        pt_pool = ctx.enter_context(
            tc.tile_pool(name="pt_acc", bufs=1, space="PSUM"))
        pt_ps = pt_pool.tile([FE, no], f32)

        del lrelu_dve_frac
        n_act = int(round(lrelu_act_frac * ncj))

        def lrelu_engine(jc):
            # deterministic interleave of ACT / DVE chunks
            return "act" if (jc * 7919) % ncj < n_act else "dve"

        with (
            tc.tile_pool(name="adjw", bufs=2 * nic) as adjw_pool,
            tc.tile_pool(name="adjt", bufs=5, space="PSUM") as tr_pool,
            tc.tile_pool(name="epool", bufs=2) as e_pool,
            tc.tile_pool(name="upool", bufs=2 * ns) as u_pool,
        ):
            adjw = {}
            for w in range(nw):
                # adj window DMA (SWDGE cast int32 -> bf16), one per i chunk
                for ic in range(nic):
                    t = adjw_pool.tile([P, jw], bf16, tag="adjw",
                                       name=f"adjw_{w}_{ic}")
                    nc.gpsimd.dma_start(
                        t[:],
                        adj_own[ic * P:(ic + 1) * P, w * jw:(w + 1) * jw])
                    adjw[ic] = t
                for jcl in range(cpw):
                    jc = w * cpw + jcl
                    # E = leakyrelu(Wh1_rep + Wh2[:, jc])   [128 j, no i]
                    e_sb = e_pool.tile([P, no], f32, tag="e")
                    eng = lrelu_engine(jc)
                    if eng == "act":
                        nc.scalar.activation(e_sb[:], wh1_rep[:], AF.Prelu,
                                             bias=wh2_sb[:, jc:jc + 1],
                                             scale=1.0, alpha=ALPHA)
                    else:
                        # t = 0.2 * (Wh1 + Wh2) ; E = max(Wh1 + Wh2, t)
                        nc.vector.tensor_scalar(e_sb[:], wh1_rep[:],
                                                wh2_sb[:, jc:jc + 1], ALPHA,
                                                AluOpType.add, AluOpType.mult)
                        nc.vector.scalar_tensor_tensor(
                            e_sb[:], wh1_rep[:], wh2_sb[:, jc:jc + 1],
                            e_sb[:], AluOpType.add, AluOpType.max)
                    # E = exp(E)
                    nc.scalar.activation(e_sb[:], e_sb[:], AF.Exp)
                    # adj.T for this j chunk, per i-split, then U = E * adj.T
                    for s in range(ns):
                        sw = min(S, no - s * S)
                        at_ps = tr_pool.tile([P, S], bf16, tag="adjt")
                        for q in range(sw // P):
                            ic = (s * S) // P + q
                            nc.tensor.transpose(
                                at_ps[:, q * P:(q + 1) * P],
                                adjw[ic][:, jcl * P:(jcl + 1) * P],
                                identity_bf[:])
                        u_sb = u_pool.tile([P, S], u_dt, tag="u")
                        nc.vector.tensor_tensor(
                            u_sb[:, 0:sw], e_sb[:, s * S:s * S + sw],
                            at_ps[:, 0:sw], AluOpType.mult)
                        nc.tensor.matmul(pt_ps[:, s * S:s * S + sw],
                                         h_ext[:, jc, :], u_sb[:, 0:sw],
                                         start=(jc == 0),
                                         stop=(jc == ncj - 1))

        # ---- phase 3: out = elu(P[:, :64] / P[:, 64]) --------------------
        pt_sb = consts.tile([FE, no], f32)
        nc.vector.tensor_copy(pt_sb[:], pt_ps[:])
        out_sb = consts.tile([P, nic, F], f32)
        with tc.tile_pool(name="fin_ps", bufs=2, space="PSUM") as fin_ps, \
             tc.tile_pool(name="fin_sb", bufs=4) as fin_sb:
            for ic in range(nic):
                ptp = fin_ps.tile([P, FE], f32, tag="ptp")
                nc.tensor.transpose(ptp[:], pt_sb[:, ic * P:(ic + 1) * P],
                                    identity[0:FE, 0:FE])
                rec = fin_sb.tile([P, 1], f32, tag="rec")
                nc.vector.reciprocal(rec[:], ptp[:, F:FE])
                hp = fin_sb.tile([P, F], f32, tag="hp")
                nc.vector.tensor_scalar(hp[:], ptp[:, 0:F], rec[:], None,
                                        AluOpType.mult)
                # elu(x) = max(x,0) + exp(min(x,0)) - 1
                mn = fin_sb.tile([P, F], f32, tag="mn")
                nc.vector.tensor_scalar(mn[:], hp[:], 0.0, None, AluOpType.min)
                nc.scalar.activation(mn[:], mn[:], AF.Exp)
                nc.vector.tensor_scalar(hp[:], hp[:], 0.0, None, AluOpType.max)
                nc.vector.scalar_tensor_tensor(
                    out_sb[:, ic, :], mn[:], 1.0, hp[:],
                    AluOpType.subtract, AluOpType.add)
        nc.sync.dma_start(out_d.ap().rearrange("(c p) f -> p c f", p=P),
                          out_sb[:])

    nc.compile()
    return nc


_CACHE = {}


def _get_program(nt, no, jw, **kw):
    key = (nt, no, jw, tuple(sorted(kw.items())))
    if key not in _CACHE:
        _CACHE[key] = build_program(nt, no, jw, **kw)
    return _CACHE[key]


def kernel(input, adj, W, a):
    from concourse.bass_utils import run_bass_kernel_spmd

    input = np.ascontiguousarray(input, dtype=np.float32)
    adj = np.ascontiguousarray(adj, dtype=np.int32)
    W = np.ascontiguousarray(W, dtype=np.float32)
    a = np.ascontiguousarray(a, dtype=np.float32)

    nt = input.shape[0]
    no = nt // N_CORES
    nc = _get_program(nt, no, 2048)

    in_maps = []
    for c in range(N_CORES):
        in_maps.append({
            "input": input,
            "input_own": input[c * no:(c + 1) * no],
            "adj_own": adj[c * no:(c + 1) * no],
            "W": W,
            "a": a,
        })
    res = run_bass_kernel_spmd(nc, in_maps, list(range(N_CORES)))
    return np.concatenate([r["out"] for r in res.results], axis=0)
